# revision 1
# baseline (speedup 1.0000x reference)
"""HGCN layer kernel for Trainium2, 8 NeuronCores, SPMD with hidden collectives.

Reference computation (N=6144, D=512):
    type_sum_a = adj_a @ x ; type_sum_b = adj_b @ x
    attn_a = sigmoid(cat[ts_a, x] @ Wa.T + ba) ; attn_b likewise
    h = x @ W_sa ; s_l = h @ a_sa[:512] ; s_r = h @ a_sa[512:]
    scores[i,j] = s_l[i] + s_r[j]
    e = adj_a * exp(-leaky_relu(scores, 0.01)) ; attn = e / (rowsum(e)+1e-5)
    x_a = attn @ h ; x_b = adj_b @ (x @ W_gcnb) + b_gcnb
    out = sigmoid(attn_a * x_a + attn_b * x_b)

Kernel strategy (per core, NL=768 local rows):
  - R = [W_sa | W_gcnb | W_sa@a_l | W_sa@a_r | Wa1.T | Wb1.T | Wa2.T | Wb2.T]
    Phase A: each core computes HX = x_local @ R for its 6 row tiles only.
    Gates reassociate (adj@x)@W1.T -> adj@(x@W1.T).
  - Attention branch stays row-sharded: AllGather [h | stats] bf16
    (0.8MB/rank -> 6.4MB out) runs DURING the (AG-independent) GCN phase.
  - GCN branch is contraction(j)-sharded: each core computes partial x_b
    for ALL rows from xw_local, plus a gb gate partial row; a bf16
    ReduceScatter sums partials and lands exactly this core's row block.
    The RS runs DURING the attention phase.
  - On this platform collectives move data through the same in-order DMA
    queues as kernel DMAs, so a collective burst head-of-line blocks later
    DMAs. Counters: resident prefetch buffers sized to ride out each burst,
    big-line chunked adjacency layouts ([8, 128, 6*768], 9KB lines) to keep
    descriptor-generation cheap, and collective-waiting loads placed after
    each phase's streaming DMAs in queue order.
  - e computed in transposed layout [j(part), i(free)]; rowsum/ga via
    zero-padded M=2 side-pass; float32r matmuls in phase A; bf16 elsewhere.
"""

import numpy as np
from contextlib import ExitStack

import concourse.bass as bass
import concourse.bacc as bacc
import concourse.mybir as mybir
import concourse.tile as tile

F32 = mybir.dt.float32
F32R = mybir.dt.float32r
BF16 = mybir.dt.bfloat16
AF = mybir.ActivationFunctionType
ALU = mybir.AluOpType

N_CORES = 8


def _chunks(total, size=512):
    out = []
    o = 0
    while o < total:
        out.append((o, min(size, total - o)))
        o += size
    return out


def build_program(n, d, nl, ba, bb, dt_a=F32R, dt_bc=BF16):
    """Build the SPMD Bass program. Returns nc."""
    JT = n // 128   # global node tiles
    IT = nl // 128  # local row tiles
    KT = d // 128   # feature k tiles
    CH = JT // IT   # adjacency chunks (= N_CORES for this shape)
    NR = 2 * d + 8  # columns of R
    NG = d + 8      # AllGather payload cols per tile: [h | stats]
    # stats cols: 0=s_l 1=s_r 2=zero 3=va 4=vb 5=wa2x 6=wb2x 7=pad

    PRE_C = 3  # attention adjacency chunks resident-prefetched (RS burst)

    nc = bacc.Bacc("TRN2", target_bir_lowering=False, debug=False,
                   num_devices=N_CORES)

    xt_dram = nc.dram_tensor("xt", [IT, KT, 128, 128], dt_a, kind="ExternalInput")
    r_dram = nc.dram_tensor("rmat", [KT, 128, NR], dt_a, kind="ExternalInput")
    adjat_dram = nc.dram_tensor("adjat3", [CH, 128, IT * nl], dt_bc,
                                kind="ExternalInput")
    adjbt_dram = nc.dram_tensor("adjbt3", [CH, 128, IT * nl], dt_bc,
                                kind="ExternalInput")
    bbias_dram = nc.dram_tensor("bbias", [128, d], F32, kind="ExternalInput")
    ident_dram = nc.dram_tensor("ident", [128, 128], F32, kind="ExternalInput")
    out_dram = nc.dram_tensor("out", [nl, d], F32, kind="ExternalOutput")

    # collective buffers
    cch_in = nc.dram_tensor("cch_in", [128, IT, NG], dt_bc)
    cch_out = nc.dram_tensor("cch_out", [N_CORES, 128, IT, NG], dt_bc,
                             addr_space="Shared")
    ccb_in = nc.dram_tensor("ccb_in", [JT, 128, d], dt_bc)
    ccb_out = nc.dram_tensor("ccb_out", [IT, 128, d], dt_bc)
    ccg_in = nc.dram_tensor("ccg_in", [N_CORES, nl], dt_bc)
    ccg_out = nc.dram_tensor("ccg_out", [1, nl], dt_bc)

    def mm(out, lhsT, rhs, start, stop, skip_group_check=False):
        nc.tensor.matmul(out, lhsT, rhs, start=start, stop=stop,
                         skip_group_check=skip_group_check)

    with tile.TileContext(nc) as tc, ExitStack() as ctx:
        const = ctx.enter_context(tc.tile_pool(name="const", bufs=1))

        ata_pre = const.tile([128, PRE_C, IT * nl], dt_bc, tag="atapre")
        hxs_lo = const.tile([128, JT // 2, NG], dt_bc, tag="hxslo")
        stats_loc = const.tile([128, IT * 8], F32, tag="statsl")
        statsl_r = const.tile([128, IT * 8], dt_bc, tag="statslr")
        sr_f32 = const.tile([128, JT], F32, tag="srf")
        slb_sb = const.tile([128, nl], F32, tag="slb")
        bbias_sb = const.tile([128, d], F32, tag="bbias")
        ident_sb = const.tile([128, 128], F32, tag="ident")
        onespad = const.tile([128, 2], dt_bc, tag="onespad")
        onespad_f = const.tile([128, 2], F32, tag="onespadf")
        ones_row = const.tile([1, 128], F32, tag="ones_r")
        neg1 = const.tile([128, 1], F32, tag="neg1")
        ba_sb = const.tile([128, 1], F32, tag="ba")
        bb_sb = const.tile([128, 1], F32, tag="bb")
        sl_row = const.tile([1, nl], F32, tag="sl_row")
        g_sb = const.tile([128, 3 * IT], F32, tag="g")  # rs|ga|gb cols
        rg_rows = const.tile([2, nl], F32, tag="rg_rows")  # row0=rs row1=ga
        gb_row = const.tile([1, nl], F32, tag="gb_row")
        gate_sb = const.tile([128, 4 * IT], F32, tag="gate")
        # gate_sb cols: [0:IT]=recip(rowsum), [IT:2IT]=sig_a, [2IT:3IT]=sig_b,
        # [3IT:4IT]=scratch

        nc.sync.dma_start(out=bbias_sb[:], in_=bbias_dram[:])
        nc.sync.dma_start(out=ident_sb[:], in_=ident_dram[:])
        nc.vector.memset(onespad_f[:], 0.0)
        nc.vector.memset(onespad_f[:, 0:1], 1.0)
        nc.vector.tensor_copy(onespad[:], onespad_f[:])
        nc.vector.memset(ones_row[:], 1.0)
        nc.vector.memset(neg1[:], -1.0)
        nc.vector.memset(ba_sb[:], float(ba))
        nc.vector.memset(bb_sb[:], float(bb))

        # ---- Phase A: local HX = x_loc @ R; stage [h|stats] for AllGather ----
        # adjBpre holds ALL of phase B's adjacency, loaded before the AG
        # trigger so B never waits on a DMA ring the collective owns; scoped
        # to phases A+B so its SBUF frees for phase C (closed after B)
        abp_cm = tc.tile_pool(name="adjBpre", bufs=1)
        abp = abp_cm.__enter__()
        atg_pre = abp.tile([128, CH, IT * nl], dt_bc, tag="atgpre")
        xwl_sb = abp.tile([128, IT * d], dt_bc, tag="xwl")
        grow_all = abp.tile([1, CH * nl], dt_bc, tag="growall")
        with tc.tile_pool(name="scopeA", bufs=1) as sca, \
             tc.tile_pool(name="xt_pool", bufs=3) as xtp, \
             tc.tile_pool(name="stageA", bufs=4) as stp, \
             tc.tile_pool(name="psA", bufs=2, space="PSUM") as psA:
            r_sb = sca.tile([128, KT, NR], dt_a, tag="r")
            nc.sync.dma_start(out=r_sb[:, 0, :], in_=r_dram[0])
            for m in range(IT):
                xt_t = xtp.tile([128, KT * 128], dt_a, tag="xt")
                for k in range(KT):
                    nc.sync.dma_start(out=xt_t[:, k * 128:(k + 1) * 128],
                                      in_=xt_dram[m, k])
                if m == 0:
                    # first matmul needs only r[0] + xt[0,0]; the rest of R
                    # queues behind the first xt tile
                    for k in range(1, KT):
                        nc.sync.dma_start(out=r_sb[:, k, :], in_=r_dram[k])
                ph = psA.tile([128, d], F32, tag="ph")
                pw = psA.tile([128, d], F32, tag="pw")
                ps = psA.tile([128, 8], F32, tag="ps")
                for k in range(KT):
                    lhsT = xt_t[:, k * 128:(k + 1) * 128]
                    st, sp = (k == 0), (k == KT - 1)
                    mm(ph[:], lhsT, r_sb[:, k, 0:d], st, sp)
                    mm(pw[:], lhsT, r_sb[:, k, d:2 * d], st, sp)
                    mm(ps[:], lhsT, r_sb[:, k, 2 * d:NR], st, sp)
                stage = stp.tile([128, NG], dt_bc, tag="stage")
                nc.scalar.copy(stage[:, 0:d], ph[:])
                nc.vector.tensor_copy(stage[:, d:NG], ps[:])
                nc.scalar.copy(xwl_sb[:, m * d:(m + 1) * d], pw[:])
                nc.vector.tensor_copy(stats_loc[:, m * 8:(m + 1) * 8], ps[:])
                nc.vector.tensor_copy(statsl_r[:, m * 8:(m + 1) * 8], ps[:])
                nc.scalar.dma_start(out=cch_in[:, m, :], in_=stage[:])

            # resident prefetches: issued behind A's own loads but before the
            # AG trigger, so they enter the DMA rings ahead of the
            # collective's pre-staged descriptors and complete early
            for g in range(CH):
                nc.sync.dma_start(out=atg_pre[:, g, :], in_=adjbt_dram[g])
            for c in range(PRE_C):
                nc.sync.dma_start(out=ata_pre[:, c, :], in_=adjat_dram[c])

        nc.gpsimd.collective_compute(
            "AllGather", mybir.AluOpType.bypass,
            replica_groups=[list(range(N_CORES))],
            ins=[cch_in[:]], outs=[cch_out[:]])

        # ---- Phase A2: build SL broadcast [128, nl] from local s_l ----
        with tc.tile_pool(name="psA2", bufs=1, space="PSUM") as psA2:
            ch = _chunks(nl)
            ptrs = [psA2.tile([1, c[1]], F32, tag=f"psl{ci}",
                              name=f"psl{ci}")
                    for ci, c in enumerate(ch)]
            for t in range(IT):
                ci, off = divmod(t * 128, 512)
                # transpose stats col (s_l of local tile t) -> row chunk
                nc.tensor.matmul(ptrs[ci][0:1, off:off + 128],
                                 stats_loc[:, t * 8:t * 8 + 1],
                                 ident_sb[:], start=True, stop=True)
            for ci, (o, w) in enumerate(ch):
                nc.vector.tensor_copy(sl_row[0:1, o:o + w], ptrs[ci][0:1, :])
            for ci, (o, w) in enumerate(ch):
                pb = psA2.tile([128, w], F32, tag="pslb")
                nc.tensor.matmul(pb[:], ones_row[:], sl_row[0:1, o:o + w],
                                 start=True, stop=True)
                nc.vector.tensor_copy(slb_sb[:, o:o + w], pb[:])

        # ---- Phase B: GCN partials for ALL rows from local xw ----
        # adjbt3[ig] = [128(local j node), IT x nl] column-chunk ig of
        # adj_b[:, local].T. For row-chunk ig (== destination rank),
        # accumulate over the 6 local j tiles; gb partial via tiny-M pass.
        with tc.tile_pool(name="stageB", bufs=42) as stb, \
             tc.tile_pool(name="psB", bufs=1, space="PSUM") as psB:
            # 6 accumulators split into two half-sets of 3 banks that
            # ping-pong across chunk boundaries: half h of chunk g+1 starts
            # while half 1-h of chunk g is still being copied out
            pbs = [psB.tile([128, d], F32, tag=f"pb{i}", name=f"pb{i}")
                   for i in range(IT)]
            chn = _chunks(nl)
            pgr = [psB.tile([1, c[1]], F32, tag=f"pg{ci}", name=f"pg{ci}")
                   for ci, c in enumerate(chn)]
            H = IT // 2
            for ig in range(CH):
                atg = atg_pre[:, ig, :]
                for half in range(2):
                    i0 = half * H
                    for j in range(IT):
                        st, sp = (j == 0), (j == IT - 1)
                        for i in range(i0, i0 + H):
                            mm(pbs[i][:],
                               atg[:, j * nl + i * 128:j * nl + (i + 1) * 128],
                               xwl_sb[:, j * d:(j + 1) * d], st, sp)
                        if half == 1:
                            vb = statsl_r[:, j * 8 + 4:j * 8 + 5]
                            for ci, (o, w) in enumerate(chn):
                                mm(pgr[ci][:], vb,
                                   atg[:, j * nl + o:j * nl + o + w], st, sp)
                    for i in range(i0, i0 + H):
                        stage = stb.tile([128, d], dt_bc, tag="stb")
                        nc.vector.tensor_copy(stage[:], pbs[i][:])
                        nc.scalar.dma_start(out=ccb_in[ig * IT + i],
                                            in_=stage[:])
                for ci, (o, w) in enumerate(chn):
                    nc.vector.tensor_copy(grow_all[0:1, ig * nl + o:ig * nl + o + w],
                                          pgr[ci][0:1, :])
            nc.scalar.dma_start(out=ccg_in[:], in_=grow_all[:])

        abp_cm.__exit__(None, None, None)


        nc.gpsimd.collective_compute(
            "ReduceScatter", mybir.AluOpType.add,
            replica_groups=[list(range(N_CORES))],
            ins=[ccb_in[:]], outs=[ccb_out[:]])
        nc.gpsimd.collective_compute(
            "ReduceScatter", mybir.AluOpType.add,
            replica_groups=[list(range(N_CORES))],
            ins=[ccg_in[:]], outs=[ccg_out[:]])

        # land the AllGather result in SBUF. Placed after phase B's DMAs in
        # program (=queue) order: the AG finishes mid-B, so these never block
        # B's adjacency streams on the in-order DMA queues. Rank r's chunk
        # holds global tiles r*IT..(r+1)*IT-1, so the landing is layout-exact.
        # The high half lands in the late pool (space freed by B's teardown,
        # not needed until ~1/2 through C).
        late = ctx.enter_context(tc.tile_pool(name="late", bufs=1))
        hxs_hi = late.tile([128, JT // 2, NG], dt_bc, tag="hxshi")

        def hxs(j):
            if j < JT // 2:
                return hxs_lo[:, j, :]
            return hxs_hi[:, j - JT // 2, :]

        for r in range(N_CORES):
            dst = hxs_lo if r < N_CORES // 2 else hxs_hi
            ro = r * IT if r < N_CORES // 2 else (r - N_CORES // 2) * IT
            nc.sync.dma_start(out=dst[:, ro:ro + IT, :], in_=cch_out[r])
            for m in range(IT):
                nc.vector.tensor_copy(sr_f32[:, r * IT + m:r * IT + m + 1],
                                      dst[:, ro + m, d + 1:d + 2])

        # ---- Phase C: e = adj_a * exp(-lrelu(s)); y_a = e^T.T @ h ----
        # rowsum and ga are row-oriented with zero-padded M=2 weights:
        # pass1 lhsT=[1|0] rhs=e -> row0 += rowsum; pass2 lhsT=[0|va]
        # rhs=adj -> row1 += ga. Disjoint rows of one accumulator pair.
        xbl_sb = late.tile([128, IT * d], dt_bc, tag="xbl")
        xa_sb = late.tile([128, IT * d], F32, tag="xa")
        gbl_bf = late.tile([1, nl], dt_bc, tag="gblbf")
        with tc.tile_pool(name="adjC", bufs=CH - PRE_C) as adjp, \
             tc.tile_pool(name="ewC", bufs=4) as ewp, \
             tc.tile_pool(name="psC", bufs=1, space="PSUM") as psC:
            ata_res = []
            for chk in range(PRE_C, CH):
                ata_t = adjp.tile([128, IT * nl], dt_bc, tag="adj")
                nc.sync.dma_start(out=ata_t[:], in_=adjat_dram[chk])
                ata_res.append(ata_t)
            pc_acc = [psC.tile([128, d], F32, tag=f"pc{i}", name=f"pc{i}")
                      for i in range(IT)]
            chn = _chunks(nl)
            prg = [psC.tile([2, c[1]], F32, tag=f"prg{ci}", name=f"prg{ci}")
                   for ci, c in enumerate(chn)]
            for chk in range(CH):
                if chk < PRE_C:
                    ata = ata_pre[:, chk, :]
                else:
                    ata = ata_res[chk - PRE_C][:]
                for jj in range(IT):
                    j = chk * IT + jj
                    at = ata[:, jj * nl:(jj + 1) * nl]
                    s_r = sr_f32[:, j:j + 1]
                    m_t = ewp.tile([128, nl], F32, tag="m")
                    nc.vector.tensor_scalar_add(m_t[:], slb_sb[:], s_r)
                    nc.vector.scalar_tensor_tensor(m_t[:], m_t[:], 0.01, m_t[:],
                                                   op0=ALU.mult, op1=ALU.max)
                    # w = exp(-m), in place
                    nc.scalar.activation(m_t[:], m_t[:], AF.Exp, scale=neg1[:])
                    e_t = ewp.tile([128, nl], dt_bc, tag="e")
                    nc.vector.tensor_tensor(e_t[:], m_t[:], at, op=ALU.mult)
                    st, sp = (j == 0), (j == JT - 1)
                    h_t = hxs(j)[:, 0:d]
                    zva = hxs(j)[:, d + 2:d + 4]
                    for i in range(IT):
                        mm(pc_acc[i][:], e_t[:, i * 128:(i + 1) * 128], h_t,
                           st, sp)
                    for ci, (o, w) in enumerate(chn):
                        mm(prg[ci][:], onespad[:], e_t[:, o:o + w], st, False)
                        mm(prg[ci][:], zva, at[:, o:o + w], False, sp)
            for i in range(IT):
                nc.scalar.copy(xa_sb[:, i * d:(i + 1) * d], pc_acc[i][:])
            for ci, (o, w) in enumerate(chn):
                nc.vector.tensor_copy(rg_rows[0:2, o:o + w], prg[ci][0:2, :])

        # land RS results (the RS finishes mid-C; placed after C's DMAs so
        # the collective wait never blocks C's adjacency streams)
        nc.sync.dma_start(out=xbl_sb[:],
                          in_=ccb_out[:].rearrange("t p c -> p t c"))
        nc.sync.dma_start(out=gbl_bf[:], in_=ccg_out[:])
        # fold the gcn bias in as soon as xbl lands (mid-C, vector slack),
        # removing one serial vector op per tile from the D critical path
        for i in range(IT):
            nc.vector.tensor_tensor(xbl_sb[:, i * d:(i + 1) * d],
                                    xbl_sb[:, i * d:(i + 1) * d],
                                    bbias_sb[:], op=ALU.add)
        nc.vector.tensor_copy(gb_row[:], gbl_bf[:])

        # ---- Phase D: transpose stat rows to columns, gates, combine ----
        with tc.tile_pool(name="psD", bufs=1, space="PSUM") as psD, \
             tc.tile_pool(name="outD", bufs=2) as outp:
            pT = psD.tile([128, 3 * IT], F32, tag="pT")
            for i in range(IT):
                # transpose [rs; ga] pair: K=2 against 2x2 identity
                nc.tensor.matmul(pT[:, 2 * i:2 * i + 2],
                                 rg_rows[0:2, i * 128:(i + 1) * 128],
                                 ident_sb[0:2, 0:2], start=True, stop=True)
                nc.tensor.matmul(pT[:, 2 * IT + i:2 * IT + i + 1],
                                 gb_row[0:1, i * 128:(i + 1) * 128],
                                 ones_row[0:1, 0:1], start=True, stop=True)
            nc.vector.tensor_copy(g_sb[:], pT[:])
            # batched gate math over all IT columns via strided views
            scr = gate_sb[:, 3 * IT:4 * IT]
            # recip(rowsum + 1e-5)
            nc.vector.tensor_scalar_add(scr, g_sb[:, 0:2 * IT:2], 1e-5)
            nc.vector.reciprocal(gate_sb[:, 0:IT], scr)
            # sig_a = sigmoid(ga + wa2x + ba)
            nc.vector.tensor_tensor(scr, g_sb[:, 1:2 * IT:2],
                                    stats_loc[:, 5:8 * IT:8], op=ALU.add)
            nc.scalar.activation(gate_sb[:, IT:2 * IT], scr,
                                 AF.Sigmoid, bias=ba_sb[:])
            # sig_b = sigmoid(gb + wb2x + bb)
            nc.vector.tensor_tensor(scr, g_sb[:, 2 * IT:3 * IT],
                                    stats_loc[:, 6:8 * IT:8], op=ALU.add)
            nc.scalar.activation(gate_sb[:, 2 * IT:3 * IT], scr,
                                 AF.Sigmoid, bias=bb_sb[:])
            for i in range(IT):
                u_t = outp.tile([128, d], F32, tag="u")
                # u = sig_a * (x_a_raw * recip)
                nc.vector.tensor_scalar(u_t[:], xa_sb[:, i * d:(i + 1) * d],
                                        gate_sb[:, i:i + 1],
                                        gate_sb[:, IT + i:IT + i + 1],
                                        op0=ALU.mult, op1=ALU.mult)
                t_t = outp.tile([128, d], F32, tag="t")
                # y = sigmoid((x_b_raw + b_gcnb) * sig_b + u); bias was
                # folded into xbl when it landed
                nc.vector.scalar_tensor_tensor(t_t[:],
                                               xbl_sb[:, i * d:(i + 1) * d],
                                               gate_sb[:, 2 * IT + i:2 * IT + i + 1],
                                               u_t[:], op0=ALU.mult, op1=ALU.add)
                y_t = outp.tile([128, d], F32, tag="y")
                nc.scalar.activation(y_t[:], t_t[:], AF.Sigmoid)
                nc.sync.dma_start(out=out_dram[i * 128:(i + 1) * 128, :],
                                  in_=y_t[:])

    nc.compile()
    return nc


def make_r_matrix(W_sa, a_sa, W_gcnb, Wa, Wb, d):
    cols = np.zeros((d, 8), dtype=np.float32)
    cols[:, 0] = W_sa @ a_sa[0, :d]
    cols[:, 1] = W_sa @ a_sa[0, d:]
    # col 2 stays zero (zero-pad for the [0|va] gate weight pair)
    cols[:, 3] = Wa[0, :d]
    cols[:, 4] = Wb[0, :d]
    cols[:, 5] = Wa[0, d:]
    cols[:, 6] = Wb[0, d:]
    return np.ascontiguousarray(
        np.concatenate([W_sa, W_gcnb, cols], axis=1)).astype(np.float32)


def make_core_inputs(x, adj_a, adj_b, R, b_gcnb, n, d, nl, core,
                     np_a=np.float32, np_bc=None):
    if np_bc is None:
        import ml_dtypes
        np_bc = ml_dtypes.bfloat16
    JT, KT, IT = n // 128, d // 128, nl // 128
    CH = JT // IT
    rows = np.arange(core * nl, (core + 1) * nl)
    xt = np.ascontiguousarray(
        x[rows].reshape(IT, 128, KT, 128).transpose(0, 2, 3, 1))
    # adjat3[ch, p, j, c] = adj_a[rows][ch*IT*128 ... , :].T grouped so one
    # chunk is a single 128-line DMA with IT*nl-wide lines
    adjat3 = np.ascontiguousarray(
        adj_a[rows].T.reshape(CH, IT, 128, nl).transpose(0, 2, 1, 3))
    # adjbt3[ig, p, j, c] = adj_b[ig*nl + c, rows[j*128 + p]]
    adjbt3 = np.ascontiguousarray(
        adj_b[:, rows].T.reshape(IT, 128, CH, nl).transpose(2, 1, 0, 3))
    return {
        "xt": xt.astype(np_a),
        "rmat": R.reshape(KT, 128, 2 * d + 8).astype(np_a),
        "adjat3": adjat3.reshape(CH, 128, IT * nl).astype(np_bc),
        "adjbt3": adjbt3.reshape(CH, 128, IT * nl).astype(np_bc),
        "bbias": np.ascontiguousarray(
            np.broadcast_to(b_gcnb, (128, d))).astype(np.float32),
        "ident": np.eye(128, dtype=np.float32),
    }


_CACHE = {}


def _install_ntff_hook():
    """Dev-only: register the axon NTFF profile hook so trace=True works.

    The agent image's antenv package lacks axon_hooks; synthesize it and
    wire trn_boot's ctypes-based hook to /opt/axon/libaxon_pjrt.so.
    """
    import sys
    import types
    try:
        from antenv import axon_hooks  # noqa: F401
        return
    except ImportError:
        pass
    import antenv
    mod = types.ModuleType("antenv.axon_hooks")
    _h = [None]
    mod.get_axon_ntff_profile_hook = lambda: _h[0]
    mod.set_axon_ntff_profile_hook = lambda hook: _h.__setitem__(0, hook)
    sys.modules["antenv.axon_hooks"] = mod
    antenv.axon_hooks = mod
    from trn_agent_boot.trn_boot import _ntff_profile_via_ctypes
    mod.set_axon_ntff_profile_hook(
        _ntff_profile_via_ctypes("/opt/axon/libaxon_pjrt.so"))


def kernel(x, adj_a, adj_b, W_sa, a_sa, W_gcnb, b_gcnb, Wa, ba, Wb, bb,
           _trace=False, _trace_kwargs=None):
    from concourse.bass_utils import run_bass_kernel_spmd
    if _trace:
        _install_ntff_hook()

    n, d = x.shape
    nl = n // N_CORES
    R = make_r_matrix(W_sa, a_sa, W_gcnb, Wa, Wb, d)

    key = (n, d, nl, float(ba[0]), float(bb[0]))
    if key not in _CACHE:
        _CACHE[key] = build_program(n, d, nl, float(ba[0]), float(bb[0]))
    nc = _CACHE[key]

    in_maps = [make_core_inputs(x, adj_a, adj_b, R, b_gcnb, n, d, nl, c)
               for c in range(N_CORES)]
    res = run_bass_kernel_spmd(nc, in_maps, list(range(N_CORES)),
                               trace=_trace, **(_trace_kwargs or {}))
    out = np.empty((n, d), dtype=np.float32)
    for c in range(N_CORES):
        out[c * nl:(c + 1) * nl] = res.results[c]["out"]
    if _trace:
        kernel._last_results = res
    return out



# revision 13
# speedup vs baseline: 1.1475x; 1.1475x over previous
"""HGCN layer kernel for Trainium2, 8 NeuronCores, SPMD with hidden collectives.

Reference computation (N=6144, D=512):
    type_sum_a = adj_a @ x ; type_sum_b = adj_b @ x
    attn_a = sigmoid(cat[ts_a, x] @ Wa.T + ba) ; attn_b likewise
    h = x @ W_sa ; s_l = h @ a_sa[:512] ; s_r = h @ a_sa[512:]
    scores[i,j] = s_l[i] + s_r[j]
    e = adj_a * exp(-leaky_relu(scores, 0.01)) ; attn = e / (rowsum(e)+1e-5)
    x_a = attn @ h ; x_b = adj_b @ (x @ W_gcnb) + b_gcnb
    out = sigmoid(attn_a * x_a + attn_b * x_b)

Kernel strategy (per core, NL=768 local rows), fp8-DoubleRow edition:
  - Both big spmm GEMMs run as fp8e4 DoubleRow matmuls (K=256/instr at
    0.5 cyc/row): adjacency is binary so fp8 is exact; xw rides as an
    fp8 hi + fp8 lo(x16) residual pair with a second host-side adjacency
    copy pre-scaled by 1/16 so the lo GEMM accumulates into the same
    PSUM bank (drains stay plain casts on the scalar engine).
  - e-chain: exp(-lrelu(t)) = min(exp(-s_l)exp(-s_r), exp(-0.01 t)) with
    exp(-0.01t) ~= 1-0.01t; the adjacency mask is folded host-side into
    penalty form C' = 193*adj - 192 so  e = max(min(A, C'-0.01s_l-0.01s_r), 0)
    is 2 DVE ops + 1 GpSimd op per tile, writing fp8 directly.
  - ga/gb gates: va/vb as fp8 hi/lo pairs; ga corrected algebraically
    for the penalty form: sum(va*adj) = (sum(va*C') + 192*sum(va))/193.
  - Collectives: AG1 = tiny bf16 [s_r|va] stats (unblocks the e-chain
    early), AG2 = fp8 h; gb partials ReduceScatter as [hi;lo] rows; x_b
    partials ReduceScatter bf16 as before.
  - rowsum+ga share one [4,w] PSUM accumulator: rows [rowsum, ga_hi,
    ga_lo, 0] via zero-padded DoubleRow lhsT weights.
"""

import numpy as np
from contextlib import ExitStack

import concourse.bass as bass
import concourse.bacc as bacc
import concourse.mybir as mybir
import concourse.tile as tile

F32 = mybir.dt.float32
F32R = mybir.dt.float32r
BF16 = mybir.dt.bfloat16
FP8 = mybir.dt.float8e4
AF = mybir.ActivationFunctionType
ALU = mybir.AluOpType
DR = mybir.MatmulPerfMode.DoubleRow

N_CORES = 8
PEN = 192.0  # penalty magnitude in C' = adj ? 1 : -PEN  (fp8-exact)


def _chunks(total, size=512):
    out = []
    o = 0
    while o < total:
        out.append((o, min(size, total - o)))
        o += size
    return out


def build_program(n, d, nl, ba, bb, dt_a=F32R):
    """Build the SPMD Bass program. Returns nc."""
    JT = n // 128   # global node tiles
    IT = nl // 128  # local row tiles
    KT = d // 128   # feature k tiles
    CH = JT // IT   # adjacency chunks (= N_CORES for this shape)
    NR = 2 * d + 8  # columns of R
    NP = JT // 2    # j-tile pairs

    nc = bacc.Bacc("TRN2", target_bir_lowering=False, debug=False,
                   num_devices=N_CORES)

    xt_dram = nc.dram_tensor("xt", [IT, KT, 128, 128], dt_a, kind="ExternalInput")
    r_dram = nc.dram_tensor("rmat", [KT, 128, NR], dt_a, kind="ExternalInput")
    # adj_a^T local slice in penalty form C' = 193*adj - 192, fp8
    adjat_dram = nc.dram_tensor("adjat3", [CH, 128, IT, nl], FP8,
                                kind="ExternalInput")
    # adj_b^T (contraction layout), raw 0/1 fp8 + 1/16-scaled copy
    adjbt_dram = nc.dram_tensor("adjbt3", [CH, 128, IT, nl], FP8,
                                kind="ExternalInput")
    adjbtl_dram = nc.dram_tensor("adjbt3l", [CH, 128, IT, nl], FP8,
                                 kind="ExternalInput")
    bbias_dram = nc.dram_tensor("bbias", [128, d], F32, kind="ExternalInput")
    ident_dram = nc.dram_tensor("ident", [128, 128], F32, kind="ExternalInput")
    out_dram = nc.dram_tensor("out", [nl, d], F32, kind="ExternalOutput")

    # collective buffers
    cs_in = nc.dram_tensor("cs_in", [128, IT, 4], BF16)           # AG1 stats
    cs_out = nc.dram_tensor("cs_out", [N_CORES, 128, IT, 4], BF16,
                            addr_space="Shared")
    ch_in = nc.dram_tensor("ch_in", [128, IT, d], FP8)            # AG2 h
    ch_out = nc.dram_tensor("ch_out", [N_CORES, 128, IT, d], FP8,
                            addr_space="Shared")
    ccb_in = nc.dram_tensor("ccb_in", [JT, 128, d], BF16)         # RS xb
    ccb_out = nc.dram_tensor("ccb_out", [IT, 128, d], BF16)
    ccg_in = nc.dram_tensor("ccg_in", [N_CORES, 2, nl], BF16)     # RS gb hi/lo
    ccg_out = nc.dram_tensor("ccg_out", [2, nl], BF16)

    def mm(out, lhsT, rhs, start, stop, perf_mode=None):
        nc.tensor.matmul(out, lhsT, rhs, start=start, stop=stop,
                         perf_mode=perf_mode)

    with tile.TileContext(nc) as tc, ExitStack() as ctx:
        const = ctx.enter_context(tc.tile_pool(name="const", bufs=1))

        hxs = const.tile([128, JT, d], FP8, tag="hxs")       # gathered h
        ata = const.tile([128, CH, IT, nl], FP8, tag="ata")  # adj_a C' resident
        pb_sb = const.tile([128, nl], BF16, tag="pb")        # exp(-s_l) bcast
        slb01 = const.tile([128, nl], BF16, tag="slb01")     # -0.01 s_l bcast
        stats_loc = const.tile([128, IT * 8], F32, tag="statsl")
        sb1 = const.tile([128, JT, 4], BF16, tag="sb1")      # AG1 landing
        sr_f32 = const.tile([128, JT], F32, tag="srf")
        qexp = const.tile([128, JT], F32, tag="qexp")        # exp(-s_r)
        s01 = const.tile([128, JT], F32, tag="s01")          # -0.01 s_r
        zva = const.tile([128, JT, 16], FP8, tag="zva")      # [0|va_hi|va_lo|0...]
        vasum_bc = const.tile([128, 1], F32, tag="vasum")
        bias_a = const.tile([128, 1], F32, tag="biasa")
        vbhl = const.tile([128, IT, 16], FP8, tag="vbhl")    # vb hi/lo local
        bbias_sb = const.tile([128, d], F32, tag="bbias")
        ident_sb = const.tile([128, 128], F32, tag="ident")
        ones4 = const.tile([128, 2, 16], FP8, tag="ones4")
        ones_row = const.tile([1, 128], F32, tag="ones_r")
        ones_col = const.tile([128, 1], F32, tag="ones_c")
        bb_sb = const.tile([128, 1], F32, tag="bb")
        sl_row = const.tile([1, nl], F32, tag="sl_row")
        er_row = const.tile([1, nl], F32, tag="er_row")
        m01_row = const.tile([1, nl], F32, tag="m01_row")
        g4_sb = const.tile([128, 4 * IT], F32, tag="g4")   # rs|ga_hi|ga_lo|-
        gg_sb = const.tile([128, 2 * IT], F32, tag="gg")   # gb_hi|gb_lo
        rg4 = const.tile([4, nl], F32, tag="rg4")
        gbl_bf = const.tile([2, nl], BF16, tag="gblbf")
        gbl_f = const.tile([2, nl], F32, tag="gblf")
        gate_sb = const.tile([128, 6 * IT], F32, tag="gate")
        # gate_sb cols: [0:IT]=recip(rowsum), [IT:2IT]=sig_a, [2IT:3IT]=sig_b,
        # [3IT:4IT]=scratch

        nc.sync.dma_start(out=bbias_sb[:], in_=bbias_dram[:])
        nc.sync.dma_start(out=ident_sb[:], in_=ident_dram[:])
        nc.vector.memset(ones4[:], 0.0)
        nc.vector.memset(ones4[:, :, 0:1], 1.0)
        nc.vector.memset(ones_row[:], 1.0)
        nc.vector.memset(ones_col[:], 1.0)
        nc.vector.memset(bb_sb[:], float(bb))
        nc.vector.memset(zva[:], 0.0)
        nc.vector.memset(vbhl[:], 0.0)

        # ---- Phase A: local HX = x_loc @ R; stage h (fp8) + stats (bf16) ----
        # R stats cols: 0=s_l 1=s_r 2=va 3=vb 4=wa2x 5=wb2x 6,7=0
        abp_cm = tc.tile_pool(name="adjBpre", bufs=1)
        abp = abp_cm.__enter__()
        atg = abp.tile([128, CH, IT, nl], FP8, tag="atg")
        atgl = abp.tile([128, CH, IT, nl], FP8, tag="atgl")
        xwlh = abp.tile([128, IT, d], FP8, tag="xwlh")
        xwll = abp.tile([128, IT, d], FP8, tag="xwll")
        grow2 = abp.tile([2, CH * nl], BF16, tag="grow2")
        with tc.tile_pool(name="scopeA", bufs=1) as sca, \
             tc.tile_pool(name="xt_pool", bufs=3) as xtp, \
             tc.tile_pool(name="stageA", bufs=4) as stp, \
             tc.tile_pool(name="psA", bufs=2, space="PSUM") as psA:
            r_sb = sca.tile([128, KT, NR], dt_a, tag="r")
            dva = sca.tile([128, IT], F32, tag="dva")
            dxw = sca.tile([128, d], BF16, tag="dxw")
            nc.sync.dma_start(out=r_sb[:, 0, :], in_=r_dram[0])
            for m in range(IT):
                xt_t = xtp.tile([128, KT * 128], dt_a, tag="xt")
                for k in range(KT):
                    nc.sync.dma_start(out=xt_t[:, k * 128:(k + 1) * 128],
                                      in_=xt_dram[m, k])
                if m == 0:
                    for k in range(1, KT):
                        nc.sync.dma_start(out=r_sb[:, k, :], in_=r_dram[k])
                ph = psA.tile([128, d], F32, tag="ph")
                pw = psA.tile([128, d], F32, tag="pw")
                ps = psA.tile([128, 8], F32, tag="ps")
                for k in range(KT):
                    lhsT = xt_t[:, k * 128:(k + 1) * 128]
                    st, sp = (k == 0), (k == KT - 1)
                    mm(ph[:], lhsT, r_sb[:, k, 0:d], st, sp)
                    mm(pw[:], lhsT, r_sb[:, k, d:2 * d], st, sp)
                    mm(ps[:], lhsT, r_sb[:, k, 2 * d:NR], st, sp)
                # h -> fp8 AG2 stage
                stage_h = stp.tile([128, d], FP8, tag="stage_h")
                nc.scalar.copy(stage_h[:], ph[:])
                nc.scalar.dma_start(out=ch_in[:, m, :], in_=stage_h[:])
                # stats -> bf16 AG1 stage [s_r, va, 0, 0]
                stage_s = stp.tile([128, 4], BF16, tag="stage_s")
                nc.vector.memset(stage_s[:, 2:4], 0.0)
                nc.vector.tensor_copy(stage_s[:, 0:2], ps[:, 1:3])
                nc.scalar.dma_start(out=cs_in[:, m, :], in_=stage_s[:])
                # xw -> fp8 hi + fp8 lo(x16)
                nc.scalar.copy(xwlh[:, m, :], pw[:])
                nc.vector.tensor_tensor(dxw[:], pw[:], xwlh[:, m, :],
                                        op=ALU.subtract)
                nc.scalar.activation(xwll[:, m, :], dxw[:], AF.Copy, scale=16.0)
                # local stats f32 + vb hi/lo fp8
                nc.vector.tensor_copy(stats_loc[:, m * 8:(m + 1) * 8], ps[:])
                nc.vector.tensor_copy(vbhl[:, m, 0:1], ps[:, 3:4])
                nc.vector.tensor_tensor(dva[:, m:m + 1], ps[:, 3:4],
                                        vbhl[:, m, 0:1], op=ALU.subtract)
                nc.vector.tensor_scalar(vbhl[:, m, 1:2], dva[:, m:m + 1],
                                        16.0, None, op0=ALU.mult)

            # resident prefetches for B and C (fp8 => all-resident)
            for g in range(CH):
                nc.sync.dma_start(out=atg[:, g], in_=adjbt_dram[g])
            for g in range(CH):
                nc.sync.dma_start(out=atgl[:, g], in_=adjbtl_dram[g])
            for g in range(CH):
                nc.sync.dma_start(out=ata[:, g], in_=adjat_dram[g])

        nc.gpsimd.collective_compute(
            "AllGather", mybir.AluOpType.bypass,
            replica_groups=[list(range(N_CORES))],
            ins=[cs_in[:]], outs=[cs_out[:]])
        nc.gpsimd.collective_compute(
            "AllGather", mybir.AluOpType.bypass,
            replica_groups=[list(range(N_CORES))],
            ins=[ch_in[:]], outs=[ch_out[:]])

        # ---- Phase A2: broadcast rows exp(-s_l), -0.01 s_l ----
        with tc.tile_pool(name="psA2", bufs=1, space="PSUM") as psA2:
            ch2 = _chunks(nl)
            ptrs = [psA2.tile([1, c[1]], F32, tag=f"psl{ci}", name=f"psl{ci}")
                    for ci, c in enumerate(ch2)]
            for t in range(IT):
                ci, off = divmod(t * 128, 512)
                nc.tensor.matmul(ptrs[ci][0:1, off:off + 128],
                                 stats_loc[:, t * 8:t * 8 + 1],
                                 ident_sb[:], start=True, stop=True)
            for ci, (o, w) in enumerate(ch2):
                nc.vector.tensor_copy(sl_row[0:1, o:o + w], ptrs[ci][0:1, :])
            nc.scalar.activation(er_row[:], sl_row[:], AF.Exp, scale=-1.0)
            nc.scalar.activation(m01_row[:], sl_row[:], AF.Copy, scale=-0.01)
            for ci, (o, w) in enumerate(ch2):
                pb1 = psA2.tile([128, w], F32, tag="pb1")
                nc.tensor.matmul(pb1[:], ones_row[:], er_row[0:1, o:o + w],
                                 start=True, stop=True)
                nc.vector.tensor_copy(pb_sb[:, o:o + w], pb1[:])
                pb2 = psA2.tile([128, w], F32, tag="pb2")
                nc.tensor.matmul(pb2[:], ones_row[:], m01_row[0:1, o:o + w],
                                 start=True, stop=True)
                nc.vector.tensor_copy(slb01[:, o:o + w], pb2[:])

        # ---- AG1 landing + stat prep (runs ~as soon as AG1 completes) ----
        for r in range(N_CORES):
            nc.sync.dma_start(out=sb1[:, r * IT:(r + 1) * IT, :],
                              in_=cs_out[r])
        nc.vector.tensor_copy(sr_f32[:], sb1[:, :, 0])
        nc.scalar.activation(qexp[:], sr_f32[:], AF.Exp, scale=-1.0)
        nc.vector.tensor_scalar(s01[:], sr_f32[:], -0.01, None, op0=ALU.mult)
        # zva: [0 | va_hi | va_lo | 0]
        nc.vector.tensor_copy(zva[:, :, 1], sb1[:, :, 1])
        dvag = const.tile([128, JT], F32, tag="dvag")
        nc.vector.tensor_tensor(dvag[:], sb1[:, :, 1], zva[:, :, 1],
                                op=ALU.subtract)
        nc.vector.tensor_scalar(zva[:, :, 2], dvag[:], 16.0, None,
                                op0=ALU.mult)
        # vasum = sum(va_hi + va_lo/16) broadcast to [128,1]
        vaf = const.tile([128, JT], F32, tag="vaf")
        nc.vector.scalar_tensor_tensor(vaf[:], dvag[:], 0.0, sb1[:, :, 1],
                                       op0=ALU.mult, op1=ALU.add)
        vacol = const.tile([128, 1], F32, tag="vacol")
        nc.vector.tensor_reduce(vacol[:], vaf[:], axis=mybir.AxisListType.X,
                                op=ALU.add)
        with tc.tile_pool(name="psV", bufs=1, space="PSUM") as psV:
            pv1 = psV.tile([1, 1], F32, tag="pv1")
            nc.tensor.matmul(pv1[:], vacol[:], ones_col[:], start=True,
                             stop=True)
            vs_row = const.tile([1, 1], F32, tag="vsrow")
            nc.vector.tensor_copy(vs_row[:], pv1[:])
            pv2 = psV.tile([128, 1], F32, tag="pv2")
            nc.tensor.matmul(pv2[:], ones_row[:], vs_row[:], start=True,
                             stop=True)
            nc.vector.tensor_copy(vasum_bc[:], pv2[:])
        # bias_a = ba + (PEN/(PEN+1)) * vasum
        nc.vector.tensor_scalar(bias_a[:], vasum_bc[:], PEN / (PEN + 1.0),
                                float(ba), op0=ALU.mult, op1=ALU.add)

        # ---- Phase B: GCN partials for ALL rows, fp8 DoubleRow hi+lo ----
        with tc.tile_pool(name="stageB", bufs=12) as stb, \
             tc.tile_pool(name="psB", bufs=1, space="PSUM") as psB:
            pbs = [psB.tile([128, d], F32, tag=f"pb{i}", name=f"pb{i}")
                   for i in range(IT)]
            chn = _chunks(nl)
            pgr = [psB.tile([16, c[1]], F32, tag=f"pg{ci}", name=f"pg{ci}")
                   for ci, c in enumerate(chn)]
            H = IT // 2
            NJP = IT // 2  # local j pairs
            for ig in range(CH):
                a3 = atg[:, ig]
                a3l = atgl[:, ig]
                for half in range(2):
                    i0 = half * H
                    for jp in range(NJP):
                        st, sp = (jp == 0), (jp == NJP - 1)
                        j2 = 2 * jp
                        xh2 = xwlh[:, j2:j2 + 2, :]
                        xl2 = xwll[:, j2:j2 + 2, :]
                        for i in range(i0, i0 + H):
                            la = a3[:, j2:j2 + 2, i * 128:(i + 1) * 128]
                            ll = a3l[:, j2:j2 + 2, i * 128:(i + 1) * 128]
                            mm(pbs[i][:], la, xh2, st, False, perf_mode=DR)
                            mm(pbs[i][:], ll, xl2, False, sp, perf_mode=DR)
                        if half == 1:
                            vb2 = vbhl[:, j2:j2 + 2, :]
                            for ci, (o, w) in enumerate(chn):
                                mm(pgr[ci][:], vb2, a3[:, j2:j2 + 2, o:o + w],
                                   st, sp, perf_mode=DR)
                    for i in range(i0, i0 + H):
                        stage = stb.tile([128, d], BF16, tag="stb")
                        nc.scalar.copy(stage[:], pbs[i][:])
                        nc.scalar.dma_start(out=ccb_in[ig * IT + i],
                                            in_=stage[:])
                for ci, (o, w) in enumerate(chn):
                    nc.scalar.copy(grow2[0:2, ig * nl + o:ig * nl + o + w],
                                   pgr[ci][0:2, :])
            for r in range(N_CORES):
                nc.scalar.dma_start(out=ccg_in[r],
                                    in_=grow2[0:2, r * nl:(r + 1) * nl])

        abp_cm.__exit__(None, None, None)

        nc.gpsimd.collective_compute(
            "ReduceScatter", mybir.AluOpType.add,
            replica_groups=[list(range(N_CORES))],
            ins=[ccg_in[:]], outs=[ccg_out[:]])
        nc.gpsimd.collective_compute(
            "ReduceScatter", mybir.AluOpType.add,
            replica_groups=[list(range(N_CORES))],
            ins=[ccb_in[:]], outs=[ccb_out[:]])

        # land the h AllGather (completes mid-B; placed after B's DMAs)
        for r in range(N_CORES):
            nc.sync.dma_start(out=hxs[:, r * IT:(r + 1) * IT, :],
                              in_=ch_out[r])

        # ---- Phase C: e2 pairs (fp8) -> DoubleRow attention spmm ----
        late = ctx.enter_context(tc.tile_pool(name="late", bufs=1))
        xbl_sb = late.tile([128, IT * d], BF16, tag="xbl")
        xa_sb = late.tile([128, IT * d], F32, tag="xa")
        with tc.tile_pool(name="e2C", bufs=5) as e2p, \
             tc.tile_pool(name="aC", bufs=3) as ap_, \
             tc.tile_pool(name="bC", bufs=3) as bp_, \
             tc.tile_pool(name="psC", bufs=1, space="PSUM") as psC:
            pc_acc = [psC.tile([128, d], F32, tag=f"pc{i}", name=f"pc{i}")
                      for i in range(IT)]
            chn = _chunks(nl)
            prg = [psC.tile([16, c[1]], F32, tag=f"prg{ci}", name=f"prg{ci}")
                   for ci, c in enumerate(chn)]
            for p in range(NP):
                chk, pin = divmod(p, IT // 2)
                t0 = 2 * p
                e2 = e2p.tile([128, 2, nl], FP8, tag="e2")
                for kk in range(2):
                    t = t0 + kk
                    jj = 2 * pin + kk
                    at_t = ata[:, chk, jj, :]
                    A_t = ap_.tile([128, nl], BF16, tag="A")
                    nc.vector.tensor_scalar(A_t[:], pb_sb[:],
                                            qexp[:, t:t + 1], None,
                                            op0=ALU.mult)
                    B_t = bp_.tile([128, nl], BF16, tag="B")
                    nc.vector.scalar_tensor_tensor(B_t[:], slb01[:],
                                                   s01[:, t:t + 1], at_t,
                                                   op0=ALU.add, op1=ALU.add)
                    nc.vector.scalar_tensor_tensor(e2[:, kk, :], B_t[:], 0.0,
                                                   A_t[:], op0=ALU.max,
                                                   op1=ALU.min)
                st, sp = (p == 0), (p == NP - 1)
                h2 = hxs[:, t0:t0 + 2, :]
                for i in range(IT):
                    mm(pc_acc[i][:], e2[:, :, i * 128:(i + 1) * 128], h2,
                       st, sp, perf_mode=DR)
                zva2 = zva[:, t0:t0 + 2, :]
                at2 = ata[:, chk, 2 * pin:2 * pin + 2, :]
                for ci, (o, w) in enumerate(chn):
                    mm(prg[ci][:], ones4[:], e2[:, :, o:o + w], st, False,
                       perf_mode=DR)
                    mm(prg[ci][:], zva2, at2[:, :, o:o + w], False, sp,
                       perf_mode=DR)
            for i in range(IT):
                nc.scalar.copy(xa_sb[:, i * d:(i + 1) * d], pc_acc[i][:])
            for ci, (o, w) in enumerate(chn):
                nc.vector.tensor_copy(rg4[0:4, o:o + w], prg[ci][0:4, :])

        # land RS results (finish mid/after-C; placed after C's DMAs)
        nc.sync.dma_start(out=xbl_sb[:],
                          in_=ccb_out[:].rearrange("t p c -> p t c"))
        nc.sync.dma_start(out=gbl_bf[:], in_=ccg_out[:])
        nc.vector.tensor_copy(gbl_f[:], gbl_bf[:])
        for i in range(IT):
            nc.vector.tensor_tensor(xbl_sb[:, i * d:(i + 1) * d],
                                    xbl_sb[:, i * d:(i + 1) * d],
                                    bbias_sb[:], op=ALU.add)

        # ---- Phase D: transpose stat rows to columns, gates, combine ----
        with tc.tile_pool(name="psD", bufs=1, space="PSUM") as psD, \
             tc.tile_pool(name="outD", bufs=2) as outp:
            pT = psD.tile([128, 4 * IT], F32, tag="pT")
            pTg = psD.tile([128, 2 * IT], F32, tag="pTg")
            for i in range(IT):
                nc.tensor.matmul(pT[:, 4 * i:4 * i + 4],
                                 rg4[0:4, i * 128:(i + 1) * 128],
                                 ident_sb[0:4, 0:4], start=True, stop=True)
                nc.tensor.matmul(pTg[:, 2 * i:2 * i + 2],
                                 gbl_f[0:2, i * 128:(i + 1) * 128],
                                 ident_sb[0:2, 0:2], start=True, stop=True)
            nc.vector.tensor_copy(g4_sb[:], pT[:])
            nc.vector.tensor_copy(gg_sb[:], pTg[:])
            scr = gate_sb[:, 3 * IT:4 * IT]
            ga_t = gate_sb[:, 4 * IT:5 * IT]
            gb_t = gate_sb[:, 5 * IT:6 * IT]
            # recip(rowsum + 1e-5)
            nc.vector.tensor_scalar(scr, g4_sb[:, 0:4 * IT:4], 1e-5, None,
                                    op0=ALU.add)
            nc.vector.reciprocal(gate_sb[:, 0:IT], scr)
            # ga = ga_hi + ga_lo/16 ; gb = gb_hi + gb_lo/16
            nc.vector.scalar_tensor_tensor(ga_t, g4_sb[:, 2:4 * IT:4],
                                           1.0 / 16.0, g4_sb[:, 1:4 * IT:4],
                                           op0=ALU.mult, op1=ALU.add)
            nc.vector.scalar_tensor_tensor(gb_t, gg_sb[:, 1:2 * IT:2],
                                           1.0 / 16.0, gg_sb[:, 0:2 * IT:2],
                                           op0=ALU.mult, op1=ALU.add)
            # sig_a = sigmoid(ga_t/(PEN+1) + wa2x + ba + PEN/(PEN+1)*vasum)
            nc.vector.scalar_tensor_tensor(scr, ga_t, 1.0 / (PEN + 1.0),
                                           stats_loc[:, 4:8 * IT:8],
                                           op0=ALU.mult, op1=ALU.add)
            nc.scalar.activation(gate_sb[:, IT:2 * IT], scr,
                                 AF.Sigmoid, bias=bias_a[:])
            # sig_b = sigmoid(gb + wb2x + bb)
            nc.vector.tensor_tensor(scr, gb_t,
                                    stats_loc[:, 5:8 * IT:8], op=ALU.add)
            nc.scalar.activation(gate_sb[:, 2 * IT:3 * IT], scr,
                                 AF.Sigmoid, bias=bb_sb[:])
            for i in range(IT):
                u_t = outp.tile([128, d], F32, tag="u")
                nc.vector.tensor_scalar(u_t[:], xa_sb[:, i * d:(i + 1) * d],
                                        gate_sb[:, i:i + 1],
                                        gate_sb[:, IT + i:IT + i + 1],
                                        op0=ALU.mult, op1=ALU.mult)
                t_t = outp.tile([128, d], F32, tag="t")
                nc.vector.scalar_tensor_tensor(t_t[:],
                                               xbl_sb[:, i * d:(i + 1) * d],
                                               gate_sb[:, 2 * IT + i:2 * IT + i + 1],
                                               u_t[:], op0=ALU.mult, op1=ALU.add)
                y_t = outp.tile([128, d], F32, tag="y")
                nc.scalar.activation(y_t[:], t_t[:], AF.Sigmoid)
                nc.sync.dma_start(out=out_dram[i * 128:(i + 1) * 128, :],
                                  in_=y_t[:])

    nc.compile()
    return nc


def make_r_matrix(W_sa, a_sa, W_gcnb, Wa, Wb, d):
    cols = np.zeros((d, 8), dtype=np.float32)
    cols[:, 0] = W_sa @ a_sa[0, :d]   # s_l
    cols[:, 1] = W_sa @ a_sa[0, d:]   # s_r
    cols[:, 2] = Wa[0, :d]            # va
    cols[:, 3] = Wb[0, :d]            # vb
    cols[:, 4] = Wa[0, d:]            # wa2x
    cols[:, 5] = Wb[0, d:]            # wb2x
    return np.ascontiguousarray(
        np.concatenate([W_sa, W_gcnb, cols], axis=1)).astype(np.float32)


def make_core_inputs(x, adj_a, adj_b, R, b_gcnb, n, d, nl, core,
                     np_a=np.float32):
    import ml_dtypes
    np_f8 = ml_dtypes.float8_e4m3
    JT, KT, IT = n // 128, d // 128, nl // 128
    CH = JT // IT
    rows = np.arange(core * nl, (core + 1) * nl)
    xt = np.ascontiguousarray(
        x[rows].reshape(IT, 128, KT, 128).transpose(0, 2, 3, 1))
    # adj_a^T local slice, penalty form C' = (PEN+1)*adj - PEN
    aT = adj_a[rows].T  # [N, nl]
    cpf = (aT * (PEN + 1.0) - PEN).astype(np.float32)
    adjat3 = np.ascontiguousarray(
        cpf.reshape(CH, IT, 128, nl).transpose(0, 2, 1, 3))
    # adj_b^T contraction layout + 1/16-scaled copy
    bT = adj_b[:, rows].T.reshape(IT, 128, CH, nl).transpose(2, 1, 0, 3)
    adjbt3 = np.ascontiguousarray(bT)
    adjbt3l = np.ascontiguousarray(bT * (1.0 / 16.0))
    return {
        "xt": xt.astype(np_a),
        "rmat": R.reshape(KT, 128, 2 * d + 8).astype(np_a),
        "adjat3": adjat3.reshape(CH, 128, IT, nl).astype(np_f8),
        "adjbt3": adjbt3.reshape(CH, 128, IT, nl).astype(np_f8),
        "adjbt3l": adjbt3l.reshape(CH, 128, IT, nl).astype(np_f8),
        "bbias": np.ascontiguousarray(
            np.broadcast_to(b_gcnb, (128, d))).astype(np.float32),
        "ident": np.eye(128, dtype=np.float32),
    }


_CACHE = {}


def _install_ntff_hook():
    """Dev-only: register the axon NTFF profile hook so trace=True works."""
    import sys
    import types
    try:
        from antenv import axon_hooks  # noqa: F401
        return
    except ImportError:
        pass
    import antenv
    mod = types.ModuleType("antenv.axon_hooks")
    _h = [None]
    mod.get_axon_ntff_profile_hook = lambda: _h[0]
    mod.set_axon_ntff_profile_hook = lambda hook: _h.__setitem__(0, hook)
    sys.modules["antenv.axon_hooks"] = mod
    antenv.axon_hooks = mod
    from trn_agent_boot.trn_boot import _ntff_profile_via_ctypes
    mod.set_axon_ntff_profile_hook(
        _ntff_profile_via_ctypes("/opt/axon/libaxon_pjrt.so"))


def kernel(x, adj_a, adj_b, W_sa, a_sa, W_gcnb, b_gcnb, Wa, ba, Wb, bb,
           _trace=False, _trace_kwargs=None):
    from concourse.bass_utils import run_bass_kernel_spmd
    if _trace:
        _install_ntff_hook()

    n, d = x.shape
    nl = n // N_CORES
    R = make_r_matrix(W_sa, a_sa, W_gcnb, Wa, Wb, d)

    key = (n, d, nl, float(ba[0]), float(bb[0]))
    if key not in _CACHE:
        _CACHE[key] = build_program(n, d, nl, float(ba[0]), float(bb[0]))
    nc = _CACHE[key]

    in_maps = [make_core_inputs(x, adj_a, adj_b, R, b_gcnb, n, d, nl, c)
               for c in range(N_CORES)]
    res = run_bass_kernel_spmd(nc, in_maps, list(range(N_CORES)),
                               trace=_trace, **(_trace_kwargs or {}))
    out = np.empty((n, d), dtype=np.float32)
    for c in range(N_CORES):
        out[c * nl:(c + 1) * nl] = res.results[c]["out"]
    if _trace:
        kernel._last_results = res
    return out
